# revision 8
# baseline (speedup 1.0000x reference)
"""CondInst dynamic mask head on 8 Trainium2 NeuronCores (v2: fp8 DoubleRow).

Math per instance i: x_i = [rel_i (2,HW); feats_b (8,HW)],
  h1 = relu(w0_i x_i + b0_i); h2 = relu(w1_i h1 + b1_i);
  out_i = sigmoid(w2_i h2 + b2_i).

rel_i is affine in the shared coords map -> folded into shared
X = [coords/128; feats] with per-instance Ahat_i and bias c0_i.

Kernel design (per core: batch b=core//2, L-half core%2 of 8192 px,
100 instances in 7 groups: 6x15 + 1x10, processed in 4 group-pairs):

- L0 runs as fp8e4 DoubleRow matmuls (0.5 cy/col): contraction rows carry
  value+residual fp8 pairs of both X and Ahat (Aq*Xq + Aq*Xr + Ar*Xq) which
  recovers ~bf16 precision at fp8 speed; the L0 bias (8*c0) rides two spare
  ones-rows as an fp8 value+residual pair.  An extra stationary column per
  group emits a constant 1.0 row that survives relu and becomes the
  ones-row for L1's bias.
- L1 is bf16 block-diag per group with contraction 8n+1 (row 8n = b1), so
  every evacuation is a pure relu with no bias operand -> group-PAIR
  [*, 2, 512] ops split across Act and DVE (GPSIMD cannot read PSUM).
- L2: fp8.  DoubleRow outputs must sit at PE column band 0, so pairs
  (g0,g1) and (g4,g5) run as single DoubleRow matmuls into partitions 0:32
  of two psum tiles pA/pB, and g2/g3/g6 run as plain fp8 matmuls at column
  positions 32/64 (plain mode supports tile positions).
- Sigmoid (+b2, /16) on the Act engine per chunk from pA/pB into SBUF,
  then batched DMA to HBM.

Scales: A=2*ahat vs X=4*X (p0 = 8*preact, bias 8*c0), w1s = w1/8 (bf16),
w2q = fp8(16*w2) -> p2 = 16*logit_nobias; sigmoid scale=1/16 bias=b2.
"""

import os
import sys

import numpy as np

sys.path.insert(0, "/opt/trn_rl_repo")
os.environ.setdefault("MYCRO_LOCAL_CACHE", "1")

B, K, C, H, Wd = 4, 100, 8, 128, 128
HW = H * Wd
LC = HW // 2            # 8192 px per core
WCH = 512               # px per chunk
NCH = LC // WCH         # 16 chunks
NCORE = 8
GS = [15, 15, 15, 15, 15, 15, 10]      # group sizes (7 groups, 100 inst)
GOFF = [0, 15, 30, 45, 60, 75, 90]
NPAIR = 4                              # group pairs: (0,1),(2,3),(4,5),(6,-)
SA, SX, SW2 = 8.0, 4.0, 16.0

_PROGRAM = None


def _q8(x):
    import ml_dtypes
    return np.asarray(x, np.float32).astype(ml_dtypes.float8_e4m3fn)


def _prep_inputs(seg_feat, conv_weight, ind):
    import ml_dtypes
    f8 = ml_dtypes.float8_e4m3fn
    bf16 = ml_dtypes.bfloat16

    seg_feat = np.asarray(seg_feat, dtype=np.float32)
    conv_weight = np.asarray(conv_weight, dtype=np.float32)
    ind64 = np.asarray(ind).astype(np.int64)

    cw = conv_weight.reshape(B, -1, HW)
    params = np.take_along_axis(cw, ind64[:, None, :], axis=2)  # [B,P,K]
    params = params.transpose(0, 2, 1)                           # [B,K,P]

    w0 = params[..., 0:80].reshape(B, K, C, C + 2)
    w1 = params[..., 80:144].reshape(B, K, C, C)
    w2 = params[..., 144:152].reshape(B, K, 1, C)
    b0 = params[..., 152:160]
    b1 = params[..., 160:168]
    b2 = params[..., 168:169]

    xi = (ind64 % Wd).astype(np.float32)
    yi = (ind64 // Wd).astype(np.float32)
    loc = np.stack([xi, yi], axis=-1)
    w0r = w0[..., 0:2]
    w0f = w0[..., 2:10]
    ahat = np.concatenate([-w0r, w0f], axis=-1)                  # [B,K,8,10]
    c0 = b0 + np.einsum("bkoc,bkc->bko", w0r, loc) / 128.0       # [B,K,8]

    lin = np.arange(HW, dtype=np.float32)
    coords_x = (lin % Wd) / 128.0
    coords_y = np.floor(lin / Wd) / 128.0

    A = (SA / SX) * ahat
    Aq = _q8(A).astype(np.float32)
    Ar = _q8(A - Aq).astype(np.float32)
    Bf = SA * c0
    Bq = _q8(Bf).astype(np.float32)
    Br = _q8(Bf - Bq).astype(np.float32)
    w2s16 = _q8(SW2 * w2[:, :, 0, :]).astype(np.float32)         # [B,K,8]

    in_maps = []
    for core in range(NCORE):
        b = core // 2
        sl = slice((core % 2) * LC, (core % 2) * LC + LC)

        # ---- xq [16, 2, LC] fp8: DoubleRow moving for L0
        X = np.empty((10, LC), np.float32)
        X[0] = coords_x[sl]
        X[1] = coords_y[sl]
        X[2:10] = seg_feat[b].reshape(C, HW)[:, sl]
        Xs = SX * X
        Xq = _q8(Xs).astype(np.float32)
        Xr = _q8(Xs - Xq).astype(np.float32)
        xq = np.zeros((16, 2, LC), np.float32)
        xq[0:10, 0] = Xq                  # term Aq*Xq
        xq[10:16, 0] = Xr[0:6]            # term Aq*Xr (rows 0..5)
        xq[0:4, 1] = Xr[6:10]             # term Aq*Xr (rows 6..9)
        xq[4:14, 1] = Xq                  # term Ar*Xq
        xq[14, 1] = 1.0                   # bias ones (Bq) + L1 ones source
        xq[15, 1] = 1.0                   # bias ones (Br)

        # ---- w0s [16, 2, 7*128] fp8: L0 stationaries (col 128g+8j+o;
        #      col 128g+8n emits the constant-1.0 row for L1's bias)
        w0s = np.zeros((16, 2, 7 * 128), np.float32)
        # ---- w1s [128, 7*128] bf16: L1 stationaries (row 8n = b1 row)
        w1s = np.zeros((128, 7 * 128), np.float32)
        # ---- w2s [128, 2, 256] fp8: L2 stationaries.
        #  cols 0:32    pair (g0,g1) DoubleRow: col m=inst, plane=group parity
        #  cols 32:64   pair (g4,g5) DoubleRow: col m=inst-60
        #  cols 64:96   g2 plain (plane 0), col 64+j
        #  cols 96:128  g3 plain (plane 1), col 96+j
        #  cols 128:160 g6 plain (plane 0), col 128+j
        w2s = np.zeros((128, 2, 256), np.float32)

        for g in range(7):
            n = GS[g]
            for j in range(n):
                i = GOFF[g] + j
                m = 128 * g + 8 * j
                for o in range(C):
                    w0s[0:10, 0, m + o] = Aq[b, i, o]
                    w0s[10:16, 0, m + o] = Aq[b, i, o, 0:6]
                    w0s[0:4, 1, m + o] = Aq[b, i, o, 6:10]
                    w0s[4:14, 1, m + o] = Ar[b, i, o]
                    w0s[14, 1, m + o] = Bq[b, i, o]
                    w0s[15, 1, m + o] = Br[b, i, o]
                w1s[8 * j:8 * j + 8, m:m + 8] = (w1[b, i] / SA).T
                w1s[8 * n, m:m + 8] = b1[b, i]
                t = g % 2
                if g in (0, 1):
                    w2s[8 * j:8 * j + 8, t, 15 * t + j] = w2s16[b, i]
                elif g in (4, 5):
                    w2s[8 * j:8 * j + 8, t, 32 + 15 * t + j] = w2s16[b, i]
                elif g == 2:
                    w2s[8 * j:8 * j + 8, 0, 64 + j] = w2s16[b, i]
                elif g == 3:
                    w2s[8 * j:8 * j + 8, 1, 96 + j] = w2s16[b, i]
                else:  # g == 6
                    w2s[8 * j:8 * j + 8, 0, 128 + j] = w2s16[b, i]
            # constant-1.0 L0 output row (rides the Bq ones-row)
            w0s[14, 1, 128 * g + 8 * n] = 1.0

        # sigmoid biases in pA/pB partition layout
        b2sa = np.zeros((128, 1), np.float32)
        b2sb = np.zeros((128, 1), np.float32)
        b2sa[0:30, 0] = b2[b, 0:30, 0]      # pair (g0,g1)
        b2sa[32:47, 0] = b2[b, 30:45, 0]    # g2
        b2sa[64:79, 0] = b2[b, 45:60, 0]    # g3
        b2sb[0:30, 0] = b2[b, 60:90, 0]     # pair (g4,g5)
        b2sb[32:42, 0] = b2[b, 90:100, 0]   # g6

        in_maps.append({
            "xq": xq.astype(f8),
            "w0s": w0s.astype(f8),
            "w1s": w1s.astype(bf16),
            "w2s": w2s.astype(f8),
            "b2sa": b2sa,
            "b2sb": b2sb,
        })

    return in_maps, (b2, np.asarray(ind).dtype)


def build_program():
    global _PROGRAM
    if _PROGRAM is not None:
        return _PROGRAM

    import concourse.tile as tile
    from concourse import bacc, mybir

    nc = bacc.Bacc("TRN2", target_bir_lowering=False, debug=False)
    f32 = mybir.dt.float32
    f8 = mybir.dt.float8e4
    bf16 = mybir.dt.bfloat16
    DR = mybir.MatmulPerfMode.DoubleRow
    Relu = mybir.ActivationFunctionType.Relu
    Sigmoid = mybir.ActivationFunctionType.Sigmoid

    xq_h = nc.dram_tensor("xq", [16, 2, LC], f8, kind="ExternalInput")
    w0s_h = nc.dram_tensor("w0s", [16, 2, 7 * 128], f8, kind="ExternalInput")
    w1s_h = nc.dram_tensor("w1s", [128, 7 * 128], bf16, kind="ExternalInput")
    w2s_h = nc.dram_tensor("w2s", [128, 2, 256], f8, kind="ExternalInput")
    b2sa_h = nc.dram_tensor("b2sa", [128, 1], f32, kind="ExternalInput")
    b2sb_h = nc.dram_tensor("b2sb", [128, 1], f32, kind="ExternalInput")
    out_h = nc.dram_tensor("out_shard", [100, LC], f32, kind="ExternalOutput")

    with tile.TileContext(nc) as tc:
        with (
            tc.tile_pool(name="const", bufs=1) as cpool,
            tc.tile_pool(name="h1p", bufs=3) as h1pool,
            tc.tile_pool(name="h2p", bufs=3) as h2pool,
            tc.tile_pool(name="ps", bufs=1, space="PSUM") as pspool,
        ):
            xq = cpool.tile([16, 2, LC], f8, tag="xq")
            w0s = cpool.tile([16, 2, 7 * 128], f8, tag="w0s")
            w1s = cpool.tile([128, 7 * 128], bf16, tag="w1s")
            w2s = cpool.tile([128, 2, 256], f8, tag="w2s")
            b2sa = cpool.tile([128, 1], f32, tag="b2sa")
            b2sb = cpool.tile([128, 1], f32, tag="b2sb")
            outba = cpool.tile([80, LC], f32, tag="outba")
            outbb = cpool.tile([48, LC], f32, tag="outbb")

            nc.gpsimd.dma_start(w0s[:], w0s_h[:])
            nc.gpsimd.dma_start(b2sa[:], b2sa_h[:])
            nc.gpsimd.dma_start(b2sb[:], b2sb_h[:])
            nc.gpsimd.dma_start(xq[:], xq_h[:])
            nc.gpsimd.dma_start(w1s[:], w1s_h[:])
            nc.gpsimd.dma_start(w2s[:], w2s_h[:])

            # PE p-state pre-warm on w0s while the xq/w1s DMAs land
            warm = pspool.tile([128, WCH], f32, tag="pa", bufs=1, name="warm")
            for i in range(24):
                nc.tensor.matmul(warm[0:32, 0:256], w0s[:, 0, 0:32],
                                 w0s[:, 0, 0:256], tile_position=(0, 0))

            # software pipeline over tasks s = 4*chunk + pair
            NT = NCH * NPAIR
            pw_by, h1_by, h2_by = {}, {}, {}
            pa_by, pb_by = {}, {}

            # evac engine per (kind, q): 0 = scalar(Act), 1 = vector(DVE)
            EV_H1 = [0, 1, 0, 1]
            EV_H2 = [1, 0, 1, 0]

            def evac(e, out_t, in_t):
                # pure relu: out = max(in, 0)
                if e == 0:
                    nc.scalar.activation(out_t, in_t, Relu)
                else:
                    nc.vector.tensor_scalar_max(out_t, in_t, 0.0)

            for s in range(NT + 3):
                c, q = s // NPAIR, s % NPAIR

                # ---- L0 for task s -> pw rows 0..8n (incl. ones row)
                if s < NT:
                    fl = slice(c * WCH, (c + 1) * WCH)
                    pw = pspool.tile([128, 2, WCH], f32, tag="pw", bufs=3,
                                     name=f"pw_{s}")
                    pw_by[s] = pw
                    for t in range(2):
                        g = 2 * q + t
                        if g >= 7:
                            continue
                        npart = 8 * GS[g] + 1
                        nc.tensor.matmul(
                            pw[0:npart, t, :],
                            w0s[:, :, 128 * g:128 * g + npart],
                            xq[:, :, fl],
                            perf_mode=DR,
                        )

                # ---- h1 = relu(pw) for task s-1
                if 0 <= s - 1 < NT:
                    q1 = (s - 1) % NPAIR
                    pw = pw_by[s - 1]
                    h1 = h1pool.tile([128, 2, WCH], bf16, tag="h1",
                                     name=f"h1_{s - 1}")
                    h1_by[s - 1] = h1
                    if q1 < 3:
                        evac(EV_H1[q1], h1[0:121, :, :], pw[0:121, :, :])
                    else:
                        evac(EV_H1[q1], h1[0:81, 0, :], pw[0:81, 0, :])

                # ---- L1 for task s-1 (overwrites pw rows 0..8n-1)
                if 0 <= s - 1 < NT:
                    q1 = (s - 1) % NPAIR
                    pw = pw_by[s - 1]
                    h1 = h1_by.pop(s - 1)
                    for t in range(2):
                        g = 2 * q1 + t
                        if g >= 7:
                            continue
                        n = GS[g]
                        nc.tensor.matmul(
                            pw[0:8 * n, t, :],
                            w1s[0:8 * n + 1, 128 * g:128 * g + 8 * n],
                            h1[0:8 * n + 1, t, :],
                        )

                # ---- h2 = relu(pw) for task s-2
                if 0 <= s - 2 < NT:
                    q2 = (s - 2) % NPAIR
                    pw = pw_by.pop(s - 2)
                    h2 = h2pool.tile([128, 2, WCH], f8, tag="h2",
                                     name=f"h2_{s - 2}")
                    h2_by[s - 2] = h2
                    if q2 < 3:
                        evac(EV_H2[q2], h2[0:120, :, :], pw[0:120, :, :])
                    else:
                        evac(EV_H2[q2], h2[0:80, 0, :], pw[0:80, 0, :])

                # ---- L2 for task s-2; sigmoid per chunk, DMA per 2 chunks
                if 0 <= s - 2 < NT:
                    c2, q2 = (s - 2) // NPAIR, (s - 2) % NPAIR
                    if q2 == 0:
                        pa_by[c2] = pspool.tile([128, WCH], f32, tag="pa",
                                                bufs=1, name=f"pa_{c2}")
                        pb_by[c2] = pspool.tile([128, WCH], f32, tag="pb",
                                                bufs=1, name=f"pb_{c2}")
                    pa, pb = pa_by[c2], pb_by[c2]
                    h2 = h2_by.pop(s - 2)
                    if q2 == 0:      # pair (g0,g1): DoubleRow at col band 0
                        nc.tensor.matmul(
                            pa[0:32, :], w2s[0:120, :, 0:32], h2[0:120, :, :],
                            perf_mode=DR, tile_position=(0, 0))
                    elif q2 == 1:    # g2, g3: plain fp8 at col bands 32/64
                        nc.tensor.matmul(
                            pa[32:64, :], w2s[0:120, 0, 64:96],
                            h2[0:120, 0, :], tile_position=(0, 32))
                        nc.tensor.matmul(
                            pa[64:96, :], w2s[0:120, 1, 96:128],
                            h2[0:120, 1, :], tile_position=(0, 64))
                    elif q2 == 2:    # pair (g4,g5): DoubleRow at col band 0
                        nc.tensor.matmul(
                            pb[0:32, :], w2s[0:120, :, 32:64],
                            h2[0:120, :, :],
                            perf_mode=DR, tile_position=(0, 0))
                    else:            # g6: plain fp8 at col band 32
                        nc.tensor.matmul(
                            pb[32:64, :], w2s[0:80, 0, 128:160],
                            h2[0:80, 0, :], tile_position=(0, 32))
                    if q2 == NPAIR - 1:
                        flc = slice(c2 * WCH, (c2 + 1) * WCH)
                        nc.scalar.activation(outba[0:80, flc], pa[0:80, :],
                                             Sigmoid, bias=b2sa[0:80, :],
                                             scale=1.0 / SW2)
                        nc.scalar.activation(outbb[0:48, flc], pb[0:48, :],
                                             Sigmoid, bias=b2sb[0:48, :],
                                             scale=1.0 / SW2)
                        pa_by.pop(c2)
                        pb_by.pop(c2)
                        if c2 % 2 == 1:
                            fl2 = slice((c2 - 1) * WCH, (c2 + 1) * WCH)
                            for p0_, r0_, ln, srct in (
                                    (0, 0, 30, 0), (32, 30, 15, 0),
                                    (64, 45, 15, 0), (0, 60, 30, 1),
                                    (32, 90, 10, 1)):
                                src = outba if srct == 0 else outbb
                                nc.gpsimd.dma_start(
                                    out_h[r0_:r0_ + ln, fl2],
                                    src[p0_:p0_ + ln, fl2])

    nc.compile()
    _PROGRAM = nc
    return nc


def kernel(seg_feat, conv_weight, ind):
    from concourse.bass_utils import run_bass_kernel_spmd

    in_maps, _ = _prep_inputs(seg_feat, conv_weight, ind)
    nc = build_program()
    res = run_bass_kernel_spmd(nc, in_maps, list(range(NCORE)))
    out = np.empty((B, K, HW), dtype=np.float32)
    for core in range(NCORE):
        b = core // 2
        lo = (core % 2) * LC
        out[b, :, lo:lo + LC] = res.results[core]["out_shard"]
    return out.reshape(B, K, H, Wd)


# revision 9
# speedup vs baseline: 1.0548x; 1.0548x over previous
"""CondInst dynamic mask head on 8 Trainium2 NeuronCores (v3: all-bf16).

Math per instance i: x_i = [rel_i (2,HW); feats_b (8,HW)],
  h1 = relu(w0_i x_i + b0_i); h2 = relu(w1_i h1 + b1_i);
  out_i = sigmoid(w2_i h2 + b2_i).

rel_i is affine in the shared coords map -> folded into shared
X = [coords/128; feats] with per-instance Ahat_i and bias c0_i.

Measured TRN2 matmul behavior (this silicon):
- moving streams ~1 col/cycle @2.4GHz only when the contraction partition
  count is ~>=120; K<=64 runs at ~half rate (458ns vs 252ns per 512-col MM).
- interleaving matmuls of different K-class/dtype costs ~280ns per switch;
  bf16-K121 next to fp8-DR-K120 is free, but plain-fp8 next to bf16 is not.
- fp8 DoubleRow gives NO streaming speedup here, and its outputs must sit
  at PE column band 0 -> useless for multi-block packing.
- matmuls with <=32-wide outputs at distinct 32-aligned column positions
  overlap each other almost fully.

So: EVERY matmul is plain bf16 with K=120/121 (one shape class):
- L0 per group: stationary [121, 8n+1] (rows 0-9 = Ahat, row 10 = c0 bias
  vs the ones-row of X, col 8n emits a constant 1.0 for L1's bias row),
  moving = X padded to 121 rows.  Group q3 is padded to full height.
- L1 per group: block-diag [121, 8n] with row 120(80->padded) = b1.
- L2 per group: [120, 32] stationaries into 7 32-aligned blocks of two psum
  tiles pA/pB; the 7 matmuls are emitted adjacently per chunk so they
  overlap across column positions.
- Evacuations are pure relu (no bias operand) on Act/DVE (GPSIMD cannot
  read PSUM); sigmoid+b2 on Act from pA/pB into SBUF; batched DMA out.
"""

import os
import sys

import numpy as np

sys.path.insert(0, "/opt/trn_rl_repo")
os.environ.setdefault("MYCRO_LOCAL_CACHE", "1")

B, K, C, H, Wd = 4, 100, 8, 128, 128
HW = H * Wd
LC = HW // 2            # 8192 px per core
WCH = 512               # px per chunk
NCH = LC // WCH         # 16 chunks
NCORE = 8
GS = [15, 15, 15, 15, 15, 15, 10]      # group sizes (7 groups, 100 inst)
GOFF = [0, 15, 30, 45, 60, 75, 90]
NPAIR = 4                              # tasks per chunk: (0,1),(2,3),(4,5),(6,)
# L2 output block of group g: (tile 0=pA/1=pB, partition base)
L2BLK = [(0, 0), (0, 32), (0, 64), (0, 96), (1, 0), (1, 32), (1, 64)]

_PROGRAM = None


def _prep_inputs(seg_feat, conv_weight, ind):
    import ml_dtypes
    bf16 = ml_dtypes.bfloat16

    seg_feat = np.asarray(seg_feat, dtype=np.float32)
    conv_weight = np.asarray(conv_weight, dtype=np.float32)
    ind64 = np.asarray(ind).astype(np.int64)

    cw = conv_weight.reshape(B, -1, HW)
    params = np.take_along_axis(cw, ind64[:, None, :], axis=2)  # [B,P,K]
    params = params.transpose(0, 2, 1)                           # [B,K,P]

    w0 = params[..., 0:80].reshape(B, K, C, C + 2)
    w1 = params[..., 80:144].reshape(B, K, C, C)
    w2 = params[..., 144:152].reshape(B, K, 1, C)
    b0 = params[..., 152:160]
    b1 = params[..., 160:168]
    b2 = params[..., 168:169]

    xi = (ind64 % Wd).astype(np.float32)
    yi = (ind64 // Wd).astype(np.float32)
    loc = np.stack([xi, yi], axis=-1)
    w0r = w0[..., 0:2]
    w0f = w0[..., 2:10]
    ahat = np.concatenate([-w0r, w0f], axis=-1)                  # [B,K,8,10]
    c0 = b0 + np.einsum("bkoc,bkc->bko", w0r, loc) / 128.0       # [B,K,8]

    lin = np.arange(HW, dtype=np.float32)
    coords_x = (lin % Wd) / 128.0
    coords_y = np.floor(lin / Wd) / 128.0

    in_maps = []
    for core in range(NCORE):
        b = core // 2
        sl = slice((core % 2) * LC, (core % 2) * LC + LC)

        # ---- xq [121, LC] bf16: L0 moving, padded to the fast K-class
        xq = np.zeros((121, LC), np.float32)
        xq[0] = coords_x[sl]
        xq[1] = coords_y[sl]
        xq[2:10] = seg_feat[b].reshape(C, HW)[:, sl]
        xq[10] = 1.0                     # bias row (c0) + L1 ones source

        # ---- w0s [121, 7*128] bf16 (col 128g+8j+o; col 128g+8n -> 1.0 row)
        w0s = np.zeros((121, 7 * 128), np.float32)
        # ---- w1s [121, 7*128] bf16 (row 8n = b1 ones-row)
        w1s = np.zeros((121, 7 * 128), np.float32)
        # ---- w2s [120, 256] bf16: group g at cols 32g..32g+32
        w2s = np.zeros((120, 256), np.float32)

        for g in range(7):
            n = GS[g]
            for j in range(n):
                i = GOFF[g] + j
                m = 128 * g + 8 * j
                w0s[0:10, m:m + 8] = ahat[b, i].T
                w0s[10, m:m + 8] = c0[b, i]
                w1s[8 * j:8 * j + 8, m:m + 8] = w1[b, i].T
                w1s[8 * n, m:m + 8] = b1[b, i]
                w2s[8 * j:8 * j + 8, 32 * g + j] = w2[b, i, 0]
            # constant-1.0 L0 output col (rides the bias row)
            w0s[10, 128 * g + 8 * n] = 1.0

        b2sa = np.zeros((128, 1), np.float32)
        b2sb = np.zeros((128, 1), np.float32)
        for g in range(7):
            tl, base = L2BLK[g]
            dst = b2sb if tl else b2sa
            dst[base:base + GS[g], 0] = b2[b, GOFF[g]:GOFF[g] + GS[g], 0]

        in_maps.append({
            "xq": xq.astype(bf16),
            "w0s": w0s.astype(bf16),
            "w1s": w1s.astype(bf16),
            "w2s": w2s.astype(bf16),
            "b2sa": b2sa,
            "b2sb": b2sb,
        })

    return in_maps, (b2, np.asarray(ind).dtype)


def build_program():
    global _PROGRAM
    if _PROGRAM is not None:
        return _PROGRAM

    import concourse.tile as tile
    from concourse import bacc, mybir

    nc = bacc.Bacc("TRN2", target_bir_lowering=False, debug=False)
    f32 = mybir.dt.float32
    bf16 = mybir.dt.bfloat16
    Relu = mybir.ActivationFunctionType.Relu
    Sigmoid = mybir.ActivationFunctionType.Sigmoid

    xq_h = nc.dram_tensor("xq", [121, LC], bf16, kind="ExternalInput")
    w0s_h = nc.dram_tensor("w0s", [121, 7 * 128], bf16, kind="ExternalInput")
    w1s_h = nc.dram_tensor("w1s", [121, 7 * 128], bf16, kind="ExternalInput")
    w2s_h = nc.dram_tensor("w2s", [120, 256], bf16, kind="ExternalInput")
    b2sa_h = nc.dram_tensor("b2sa", [128, 1], f32, kind="ExternalInput")
    b2sb_h = nc.dram_tensor("b2sb", [128, 1], f32, kind="ExternalInput")
    out_h = nc.dram_tensor("out_shard", [100, LC], f32, kind="ExternalOutput")

    with tile.TileContext(nc) as tc:
        with (
            tc.tile_pool(name="const", bufs=1) as cpool,
            tc.tile_pool(name="h1p", bufs=3) as h1pool,
            tc.tile_pool(name="h2p", bufs=6) as h2pool,
            tc.tile_pool(name="ps", bufs=1, space="PSUM") as pspool,
        ):
            xq = cpool.tile([121, LC], bf16, tag="xq")
            w0s = cpool.tile([121, 7 * 128], bf16, tag="w0s")
            w1s = cpool.tile([121, 7 * 128], bf16, tag="w1s")
            w2s = cpool.tile([120, 256], bf16, tag="w2s")
            b2sa = cpool.tile([128, 1], f32, tag="b2sa")
            b2sb = cpool.tile([128, 1], f32, tag="b2sb")
            outba = cpool.tile([112, LC], f32, tag="outba")
            outbb = cpool.tile([80, LC], f32, tag="outbb")

            nc.gpsimd.dma_start(w0s[:], w0s_h[:])
            nc.gpsimd.dma_start(b2sa[:], b2sa_h[:])
            nc.gpsimd.dma_start(b2sb[:], b2sb_h[:])
            nc.gpsimd.dma_start(xq[:], xq_h[:])
            nc.gpsimd.dma_start(w1s[:], w1s_h[:])
            nc.gpsimd.dma_start(w2s[:], w2s_h[:])

            # PE p-state pre-warm on w0s while the xq/w1s DMAs land
            warm = pspool.tile([128, WCH], f32, tag="pa", bufs=1, name="warm")
            for i in range(24):
                nc.tensor.matmul(warm[0:32, 0:256], w0s[:, 0:32],
                                 w0s[:, 0:256], tile_position=(0, 0))

            # software pipeline over tasks s = 4*chunk + pair
            NT = NCH * NPAIR
            pw_by, h1_by, h2_by = {}, {}, {}
            pa_by, pb_by = {}, {}

            # evac engine per q: 0 = scalar(Act), 1 = vector(DVE)
            EV_H1 = [1, 0, 1, 0]
            EV_H2 = [1, 0, 1, 0]

            def evac(e, out_t, in_t):
                # pure relu: out = max(in, 0)
                if e == 0:
                    nc.scalar.activation(out_t, in_t, Relu)
                else:
                    nc.vector.tensor_scalar_max(out_t, in_t, 0.0)

            for s in range(NT + 3):
                c, q = s // NPAIR, s % NPAIR

                # ---- L0 for task s (both groups; q3 padded to 121 rows)
                if s < NT:
                    fl = slice(c * WCH, (c + 1) * WCH)
                    pw = pspool.tile([128, 2, WCH], f32, tag="pw", bufs=3,
                                     name=f"pw_{s}")
                    pw_by[s] = pw
                    for t in range(2):
                        g = 2 * q + t
                        if g >= 7:
                            continue
                        nc.tensor.matmul(
                            pw[0:121, t, :],
                            w0s[:, 128 * g:128 * g + 121],
                            xq[:, fl],
                        )

                # ---- h1 = relu(pw) for task s-1
                if 0 <= s - 1 < NT:
                    q1 = (s - 1) % NPAIR
                    pw = pw_by[s - 1]
                    h1 = h1pool.tile([128, 2, WCH], bf16, tag="h1",
                                     name=f"h1_{s - 1}")
                    h1_by[s - 1] = h1
                    if q1 < 3:
                        evac(EV_H1[q1], h1[0:121, :, :], pw[0:121, :, :])
                    else:
                        evac(EV_H1[q1], h1[0:121, 0, :], pw[0:121, 0, :])

                # ---- L1 for task s-1 (overwrites pw; out padded to 120)
                if 0 <= s - 1 < NT:
                    q1 = (s - 1) % NPAIR
                    pw = pw_by[s - 1]
                    h1 = h1_by.pop(s - 1)
                    for t in range(2):
                        g = 2 * q1 + t
                        if g >= 7:
                            continue
                        nc.tensor.matmul(
                            pw[0:120, t, :],
                            w1s[0:121, 128 * g:128 * g + 120],
                            h1[0:121, t, :],
                        )

                # ---- h2 = relu(pw) for task s-2
                if 0 <= s - 2 < NT:
                    q2 = (s - 2) % NPAIR
                    pw = pw_by.pop(s - 2)
                    h2 = h2pool.tile([128, 2, WCH], bf16, tag="h2",
                                     name=f"h2_{s - 2}")
                    h2_by[s - 2] = h2
                    if q2 < 3:
                        evac(EV_H2[q2], h2[0:120, :, :], pw[0:120, :, :])
                    else:
                        evac(EV_H2[q2], h2[0:120, 0, :], pw[0:120, 0, :])

                # ---- L2: all 7 groups of chunk c2, batched adjacently so
                # the 32-wide matmuls overlap across column positions
                if 0 <= s - 2 < NT and (s - 2) % NPAIR == NPAIR - 1:
                    c2 = (s - 2) // NPAIR
                    pa = pspool.tile([128, WCH], f32, tag="pa", bufs=1,
                                     name=f"pa_{c2}")
                    pb = pspool.tile([128, WCH], f32, tag="pb", bufs=1,
                                     name=f"pb_{c2}")
                    for g in range(7):
                        sg, tg = 4 * c2 + g // 2, g % 2
                        h2 = h2_by[sg]
                        tl, base = L2BLK[g]
                        dst = pb if tl else pa
                        nc.tensor.matmul(
                            dst[base:base + 32, :],
                            w2s[:, 32 * g:32 * g + 32],
                            h2[0:120, tg, :],
                            tile_position=(0, base),
                        )
                    for g in range(7):
                        sg = 4 * c2 + g // 2
                        h2_by.pop(sg, None)
                    flc = slice(c2 * WCH, (c2 + 1) * WCH)
                    nc.scalar.activation(outba[0:112, flc], pa[0:112, :],
                                         Sigmoid, bias=b2sa[0:112, :],
                                         scale=1.0)
                    nc.scalar.activation(outbb[0:80, flc], pb[0:80, :],
                                         Sigmoid, bias=b2sb[0:80, :],
                                         scale=1.0)
                    if c2 % 2 == 1:
                        fl2 = slice((c2 - 1) * WCH, (c2 + 1) * WCH)
                        for g in range(7):
                            tl, base = L2BLK[g]
                            src = outbb if tl else outba
                            nc.gpsimd.dma_start(
                                out_h[GOFF[g]:GOFF[g] + GS[g], fl2],
                                src[base:base + GS[g], fl2])

    nc.compile()
    _PROGRAM = nc
    return nc


def kernel(seg_feat, conv_weight, ind):
    from concourse.bass_utils import run_bass_kernel_spmd

    in_maps, _ = _prep_inputs(seg_feat, conv_weight, ind)
    nc = build_program()
    res = run_bass_kernel_spmd(nc, in_maps, list(range(NCORE)))
    out = np.empty((B, K, HW), dtype=np.float32)
    for core in range(NCORE):
        b = core // 2
        lo = (core % 2) * LC
        out[b, :, lo:lo + LC] = res.results[core]["out_shard"]
    return out.reshape(B, K, H, Wd)


# revision 12
# speedup vs baseline: 1.3059x; 1.2380x over previous
"""CondInst dynamic mask head on 8 Trainium2 NeuronCores (v3: all-bf16).

Math per instance i: x_i = [rel_i (2,HW); feats_b (8,HW)],
  h1 = relu(w0_i x_i + b0_i); h2 = relu(w1_i h1 + b1_i);
  out_i = sigmoid(w2_i h2 + b2_i).

rel_i is affine in the shared coords map -> folded into shared
X = [coords/128; feats] with per-instance Ahat_i and bias c0_i.

Measured TRN2 matmul behavior (this silicon):
- moving streams ~1 col/cycle @2.4GHz only when the contraction partition
  count is ~>=120; K<=64 runs at ~half rate (458ns vs 252ns per 512-col MM).
- interleaving matmuls of different K-class/dtype costs ~280ns per switch;
  bf16-K121 next to fp8-DR-K120 is free, but plain-fp8 next to bf16 is not.
- fp8 DoubleRow gives NO streaming speedup here, and its outputs must sit
  at PE column band 0 -> useless for multi-block packing.
- matmuls with <=32-wide outputs at distinct 32-aligned column positions
  overlap each other almost fully.

So: EVERY matmul is plain bf16 with K=120/121 (one shape class):
- L0 per group: stationary [121, 8n+1] (rows 0-9 = Ahat, row 10 = c0 bias
  vs the ones-row of X, col 8n emits a constant 1.0 for L1's bias row),
  moving = X padded to 121 rows.  Group q3 is padded to full height.
- L1 per group: block-diag [121, 8n] with row 120(80->padded) = b1.
- L2 per group: [120, 32] stationaries into 7 32-aligned blocks of two psum
  tiles pA/pB; the 7 matmuls are emitted adjacently per chunk so they
  overlap across column positions.
- Evacuations are pure relu (no bias operand) on Act/DVE (GPSIMD cannot
  read PSUM); sigmoid+b2 on Act from pA/pB into SBUF; batched DMA out.
"""

import os
import sys

import numpy as np

sys.path.insert(0, "/opt/trn_rl_repo")
os.environ.setdefault("MYCRO_LOCAL_CACHE", "1")

B, K, C, H, Wd = 4, 100, 8, 128, 128
HW = H * Wd
LC = HW // 2            # 8192 px per core
WCH = 512               # px per chunk
NCH = LC // WCH         # 16 chunks
NCORE = 8
GS = [15, 15, 15, 15, 15, 15, 10]      # group sizes (7 groups, 100 inst)
GOFF = [0, 15, 30, 45, 60, 75, 90]
NPAIR = 4                              # tasks per chunk: (0,1),(2,3),(4,5),(6,)
# L2 output block of group g: (tile 0=pA/1=pB, partition base)
L2BLK = [(0, 0), (0, 32), (0, 64), (0, 96), (1, 0), (1, 32), (1, 64)]

_PROGRAM = None


def _prep_inputs(seg_feat, conv_weight, ind):
    import ml_dtypes
    bf16 = ml_dtypes.bfloat16

    seg_feat = np.asarray(seg_feat, dtype=np.float32)
    conv_weight = np.asarray(conv_weight, dtype=np.float32)
    ind64 = np.asarray(ind).astype(np.int64)

    cw = conv_weight.reshape(B, -1, HW)
    params = np.take_along_axis(cw, ind64[:, None, :], axis=2)  # [B,P,K]
    params = params.transpose(0, 2, 1)                           # [B,K,P]

    w0 = params[..., 0:80].reshape(B, K, C, C + 2)
    w1 = params[..., 80:144].reshape(B, K, C, C)
    w2 = params[..., 144:152].reshape(B, K, 1, C)
    b0 = params[..., 152:160]
    b1 = params[..., 160:168]
    b2 = params[..., 168:169]

    xi = (ind64 % Wd).astype(np.float32)
    yi = (ind64 // Wd).astype(np.float32)
    loc = np.stack([xi, yi], axis=-1)
    w0r = w0[..., 0:2]
    w0f = w0[..., 2:10]
    ahat = np.concatenate([-w0r, w0f], axis=-1)                  # [B,K,8,10]
    c0 = b0 + np.einsum("bkoc,bkc->bko", w0r, loc) / 128.0       # [B,K,8]

    lin = np.arange(HW, dtype=np.float32)
    coords_x = (lin % Wd) / 128.0
    coords_y = np.floor(lin / Wd) / 128.0

    in_maps = []
    for core in range(NCORE):
        b = core // 2
        sl = slice((core % 2) * LC, (core % 2) * LC + LC)

        # ---- xq [121, LC] bf16: L0 moving, padded to the fast K-class
        xq = np.zeros((121, LC), np.float32)
        xq[0] = coords_x[sl]
        xq[1] = coords_y[sl]
        xq[2:10] = seg_feat[b].reshape(C, HW)[:, sl]
        xq[10] = 1.0                     # bias row (c0) + L1 ones source

        # ---- w0s [121, 7*128] bf16 (col 128g+8j+o; col 128g+8n -> 1.0 row)
        w0s = np.zeros((121, 7 * 128), np.float32)
        # ---- w1s [121, 7*128] bf16 (row 8n = b1 ones-row)
        w1s = np.zeros((121, 7 * 128), np.float32)
        # ---- w2s [120, 256] bf16: group g at cols 32g..32g+32
        w2s = np.zeros((120, 2, 128), np.float32)

        for g in range(7):
            n = GS[g]
            for j in range(n):
                i = GOFF[g] + j
                m = 128 * g + 8 * j
                w0s[0:10, m:m + 8] = ahat[b, i].T
                w0s[10, m:m + 8] = c0[b, i]
                w1s[8 * j:8 * j + 8, m:m + 8] = w1[b, i].T
                w1s[8 * n, m:m + 8] = b1[b, i]
                u = 32 * (g // 2) + 15 * (g % 2) + j
                w2s[8 * j:8 * j + 8, g % 2, u] = w2[b, i, 0]
            # constant-1.0 L0 output col (rides the bias row)
            w0s[10, 128 * g + 8 * n] = 1.0

        b2sa = np.zeros((128, 1), np.float32)
        for blk in range(4):
            lo, n = 30 * blk, min(30, 100 - 30 * blk)
            b2sa[32 * blk:32 * blk + n, 0] = b2[b, lo:lo + n, 0]

        in_maps.append({
            "xq": xq.astype(bf16),
            "w0s": w0s.astype(bf16),
            "w1s": w1s.astype(bf16),
            "w2s": w2s.astype(bf16),
            "b2sa": b2sa,
        })

    return in_maps, (b2, np.asarray(ind).dtype)


def build_program():
    global _PROGRAM
    if _PROGRAM is not None:
        return _PROGRAM

    import concourse.tile as tile
    from concourse import bacc, mybir

    nc = bacc.Bacc("TRN2", target_bir_lowering=False, debug=False)
    f32 = mybir.dt.float32
    bf16 = mybir.dt.bfloat16
    Relu = mybir.ActivationFunctionType.Relu
    Sigmoid = mybir.ActivationFunctionType.Sigmoid

    xq_h = nc.dram_tensor("xq", [121, LC], bf16, kind="ExternalInput")
    w0s_h = nc.dram_tensor("w0s", [121, 7 * 128], bf16, kind="ExternalInput")
    w1s_h = nc.dram_tensor("w1s", [121, 7 * 128], bf16, kind="ExternalInput")
    w2s_h = nc.dram_tensor("w2s", [120, 2, 128], bf16, kind="ExternalInput")
    b2sa_h = nc.dram_tensor("b2sa", [128, 1], f32, kind="ExternalInput")
    out_h = nc.dram_tensor("out_shard", [100, LC], f32, kind="ExternalOutput")

    with tile.TileContext(nc) as tc:
        with (
            tc.tile_pool(name="const", bufs=1) as cpool,
            tc.tile_pool(name="h1p", bufs=3) as h1pool,
            tc.tile_pool(name="h2p", bufs=6) as h2pool,
            tc.tile_pool(name="ps", bufs=1, space="PSUM") as pspool,
        ):
            xq = cpool.tile([121, LC], bf16, tag="xq")
            w0s = cpool.tile([121, 7 * 128], bf16, tag="w0s")
            w1s = cpool.tile([121, 7 * 128], bf16, tag="w1s")
            w2s = cpool.tile([120, 2, 128], bf16, tag="w2s")
            b2sa = cpool.tile([128, 1], f32, tag="b2sa")
            outba = cpool.tile([106, LC], f32, tag="outba")

            nc.gpsimd.dma_start(w0s[:], w0s_h[:])
            nc.gpsimd.dma_start(b2sa[:], b2sa_h[:])
            nc.gpsimd.dma_start(xq[:], xq_h[:])
            nc.gpsimd.dma_start(w1s[:], w1s_h[:])
            nc.gpsimd.dma_start(w2s[:], w2s_h[:])

            # PE p-state pre-warm on w0s while the xq/w1s DMAs land
            warm = pspool.tile([128, WCH], f32, tag="pa", bufs=2, name="warm")
            for i in range(24):
                nc.tensor.matmul(warm[0:32, 0:256], w0s[:, 0:32],
                                 w0s[:, 0:256], tile_position=(0, 0))

            # software pipeline over tasks s = 4*chunk + pair
            NT = NCH * NPAIR
            pw_by, h1_by, h2_by = {}, {}, {}
            pa_by, pb_by = {}, {}

            # evac engine per q: 0 = scalar(Act), 1 = vector(DVE)
            EV_H1 = [1, 0, 1, 0]
            EV_H2 = [0, 1, 0, 1]

            def evac(e, out_t, in_t):
                # pure relu: out = max(in, 0)
                if e == 0:
                    nc.scalar.activation(out_t, in_t, Relu)
                else:
                    nc.vector.tensor_scalar_max(out_t, in_t, 0.0)

            for s in range(NT + 3):
                c, q = s // NPAIR, s % NPAIR

                # ---- L0 for task s (both groups; q3 padded to 121 rows)
                if s < NT:
                    fl = slice(c * WCH, (c + 1) * WCH)
                    pw = pspool.tile([128, 2, WCH], f32, tag="pw", bufs=3,
                                     name=f"pw_{s}")
                    pw_by[s] = pw
                    for t in range(2):
                        g = 2 * q + t
                        if g >= 7:
                            continue
                        nc.tensor.matmul(
                            pw[0:121, t, :],
                            w0s[:, 128 * g:128 * g + 121],
                            xq[:, fl],
                        )

                # ---- h1 = relu(pw) for task s-1
                if 0 <= s - 1 < NT:
                    q1 = (s - 1) % NPAIR
                    pw = pw_by[s - 1]
                    h1 = h1pool.tile([128, 2, WCH], bf16, tag="h1",
                                     name=f"h1_{s - 1}")
                    h1_by[s - 1] = h1
                    if q1 < 3:
                        evac(EV_H1[q1], h1[0:121, :, :], pw[0:121, :, :])
                    else:
                        evac(EV_H1[q1], h1[0:121, 0, :], pw[0:121, 0, :])

                # ---- L1 for task s-1 (overwrites pw; out padded to 120)
                if 0 <= s - 1 < NT:
                    q1 = (s - 1) % NPAIR
                    pw = pw_by[s - 1]
                    h1 = h1_by.pop(s - 1)
                    for t in range(2):
                        g = 2 * q1 + t
                        if g >= 7:
                            continue
                        nc.tensor.matmul(
                            pw[0:120, t, :],
                            w1s[0:121, 128 * g:128 * g + 120],
                            h1[0:121, t, :],
                        )

                # ---- h2 = relu(pw) for task s-2
                if 0 <= s - 2 < NT:
                    q2 = (s - 2) % NPAIR
                    pw = pw_by.pop(s - 2)
                    h2 = h2pool.tile([128, 2, WCH], bf16, tag="h2",
                                     name=f"h2_{s - 2}")
                    h2_by[s - 2] = h2
                    if q2 < 3:
                        evac(EV_H2[q2], h2[0:120, :, :], pw[0:120, :, :])
                    else:
                        evac(EV_H2[q2], h2[0:120, 0, :], pw[0:120, 0, :])

                # ---- L2: all 7 groups of chunk c2, batched; two groups
                # accumulate into each 30-wide 32-aligned block of pa
                if 0 <= s - 2 < NT and (s - 2) % NPAIR == NPAIR - 1:
                    c2 = (s - 2) // NPAIR
                    pa = pspool.tile([128, WCH], f32, tag="pa", bufs=2,
                                     name=f"pa_{c2}")
                    for g in range(7):
                        sg, tg = 4 * c2 + g // 2, g % 2
                        h2 = h2_by[sg]
                        blk = g // 2
                        nc.tensor.matmul(
                            pa[32 * blk:32 * blk + 30, :],
                            w2s[:, g % 2, 32 * blk:32 * blk + 30],
                            h2[0:120, tg, :],
                            start=(g % 2 == 0),
                            stop=(g % 2 == 1 or g == 6),
                            skip_group_check=True,
                            tile_position=(0, 32 * blk),
                        )
                    for g in range(7):
                        h2_by.pop(4 * c2 + g // 2, None)
                    flc = slice(c2 * WCH, (c2 + 1) * WCH)
                    nc.scalar.activation(outba[0:106, flc], pa[0:106, :],
                                         Sigmoid, bias=b2sa[0:106, :],
                                         scale=1.0)
                    if c2 % 2 == 1:
                        fl2 = slice((c2 - 1) * WCH, (c2 + 1) * WCH)
                        for blk in range(4):
                            lo, n = 30 * blk, min(30, 100 - 30 * blk)
                            nc.gpsimd.dma_start(
                                out_h[lo:lo + n, fl2],
                                outba[32 * blk:32 * blk + n, fl2])

    nc.compile()
    _PROGRAM = nc
    return nc


def kernel(seg_feat, conv_weight, ind):
    from concourse.bass_utils import run_bass_kernel_spmd

    in_maps, _ = _prep_inputs(seg_feat, conv_weight, ind)
    nc = build_program()
    res = run_bass_kernel_spmd(nc, in_maps, list(range(NCORE)))
    out = np.empty((B, K, HW), dtype=np.float32)
    for core in range(NCORE):
        b = core // 2
        lo = (core % 2) * LC
        out[b, :, lo:lo + LC] = res.results[core]["out_shard"]
    return out.reshape(B, K, H, Wd)


# revision 13
# speedup vs baseline: 1.3172x; 1.0087x over previous
"""CondInst dynamic mask head on 8 Trainium2 NeuronCores (v3: all-bf16).

Math per instance i: x_i = [rel_i (2,HW); feats_b (8,HW)],
  h1 = relu(w0_i x_i + b0_i); h2 = relu(w1_i h1 + b1_i);
  out_i = sigmoid(w2_i h2 + b2_i).

rel_i is affine in the shared coords map -> folded into shared
X = [coords/128; feats] with per-instance Ahat_i and bias c0_i.

Measured TRN2 matmul behavior (this silicon):
- moving streams ~1 col/cycle @2.4GHz only when the contraction partition
  count is ~>=120; K<=64 runs at ~half rate (458ns vs 252ns per 512-col MM).
- interleaving matmuls of different K-class/dtype costs ~280ns per switch;
  bf16-K121 next to fp8-DR-K120 is free, but plain-fp8 next to bf16 is not.
- fp8 DoubleRow gives NO streaming speedup here, and its outputs must sit
  at PE column band 0 -> useless for multi-block packing.
- matmuls with <=32-wide outputs at distinct 32-aligned column positions
  overlap each other almost fully.

So: EVERY matmul is plain bf16 with K=120/121 (one shape class):
- L0 per group: stationary [121, 8n+1] (rows 0-9 = Ahat, row 10 = c0 bias
  vs the ones-row of X, col 8n emits a constant 1.0 for L1's bias row),
  moving = X padded to 121 rows.  Group q3 is padded to full height.
- L1 per group: block-diag [121, 8n] with row 120(80->padded) = b1.
- L2 per group: [120, 32] stationaries into 7 32-aligned blocks of two psum
  tiles pA/pB; the 7 matmuls are emitted adjacently per chunk so they
  overlap across column positions.
- Evacuations are pure relu (no bias operand) on Act/DVE (GPSIMD cannot
  read PSUM); sigmoid+b2 on Act from pA/pB into SBUF; batched DMA out.
"""

import os
import sys

import numpy as np

sys.path.insert(0, "/opt/trn_rl_repo")
os.environ.setdefault("MYCRO_LOCAL_CACHE", "1")

B, K, C, H, Wd = 4, 100, 8, 128, 128
HW = H * Wd
LC = HW // 2            # 8192 px per core
WCH = 512               # px per chunk
NCH = LC // WCH         # 16 chunks
NCORE = 8
GS = [15, 15, 15, 15, 15, 15, 10]      # group sizes (7 groups, 100 inst)
GOFF = [0, 15, 30, 45, 60, 75, 90]
NPAIR = 4                              # tasks per chunk: (0,1),(2,3),(4,5),(6,)
# L2 output block of group g: (tile 0=pA/1=pB, partition base)
L2BLK = [(0, 0), (0, 32), (0, 64), (0, 96), (1, 0), (1, 32), (1, 64)]

_PROGRAM = None


def _prep_inputs(seg_feat, conv_weight, ind):
    import ml_dtypes
    bf16 = ml_dtypes.bfloat16

    seg_feat = np.asarray(seg_feat, dtype=np.float32)
    conv_weight = np.asarray(conv_weight, dtype=np.float32)
    ind64 = np.asarray(ind).astype(np.int64)

    cw = conv_weight.reshape(B, -1, HW)
    params = np.take_along_axis(cw, ind64[:, None, :], axis=2)  # [B,P,K]
    params = params.transpose(0, 2, 1)                           # [B,K,P]

    w0 = params[..., 0:80].reshape(B, K, C, C + 2)
    w1 = params[..., 80:144].reshape(B, K, C, C)
    w2 = params[..., 144:152].reshape(B, K, 1, C)
    b0 = params[..., 152:160]
    b1 = params[..., 160:168]
    b2 = params[..., 168:169]

    xi = (ind64 % Wd).astype(np.float32)
    yi = (ind64 // Wd).astype(np.float32)
    loc = np.stack([xi, yi], axis=-1)
    w0r = w0[..., 0:2]
    w0f = w0[..., 2:10]
    ahat = np.concatenate([-w0r, w0f], axis=-1)                  # [B,K,8,10]
    c0 = b0 + np.einsum("bkoc,bkc->bko", w0r, loc) / 128.0       # [B,K,8]

    lin = np.arange(HW, dtype=np.float32)
    coords_x = (lin % Wd) / 128.0
    coords_y = np.floor(lin / Wd) / 128.0

    in_maps = []
    for core in range(NCORE):
        b = core // 2
        sl = slice((core % 2) * LC, (core % 2) * LC + LC)

        # ---- xq [121, LC] bf16: L0 moving, padded to the fast K-class
        xq = np.zeros((121, LC), np.float32)
        xq[0] = coords_x[sl]
        xq[1] = coords_y[sl]
        xq[2:10] = seg_feat[b].reshape(C, HW)[:, sl]
        xq[10] = 1.0                     # bias row (c0) + L1 ones source

        # ---- w0s [121, 7*128] bf16 (col 128g+8j+o; col 128g+8n -> 1.0 row)
        w0s = np.zeros((121, 7 * 128), np.float32)
        # ---- w1s [121, 7*128] bf16 (row 8n = b1 ones-row)
        w1s = np.zeros((121, 7 * 128), np.float32)
        # ---- w2s [120, 256] bf16: group g at cols 32g..32g+32
        w2s = np.zeros((120, 2, 128), np.float32)

        for g in range(7):
            n = GS[g]
            for j in range(n):
                i = GOFF[g] + j
                m = 128 * g + 8 * j
                w0s[0:10, m:m + 8] = ahat[b, i].T
                w0s[10, m:m + 8] = c0[b, i]
                w1s[8 * j:8 * j + 8, m:m + 8] = w1[b, i].T
                w1s[8 * n, m:m + 8] = b1[b, i]
                u = 32 * (g // 2) + 15 * (g % 2) + j
                w2s[8 * j:8 * j + 8, g % 2, u] = w2[b, i, 0]
            # constant-1.0 L0 output col (rides the bias row)
            w0s[10, 128 * g + 8 * n] = 1.0

        b2sa = np.zeros((128, 1), np.float32)
        for blk in range(4):
            lo, n = 30 * blk, min(30, 100 - 30 * blk)
            b2sa[32 * blk:32 * blk + n, 0] = b2[b, lo:lo + n, 0]

        in_maps.append({
            "xq": xq.astype(bf16),
            "w0s": w0s.astype(bf16),
            "w1s": w1s.astype(bf16),
            "w2s": w2s.astype(bf16),
            "b2sa": b2sa,
        })

    return in_maps, (b2, np.asarray(ind).dtype)


def build_program():
    global _PROGRAM
    if _PROGRAM is not None:
        return _PROGRAM

    import concourse.tile as tile
    from concourse import bacc, mybir

    nc = bacc.Bacc("TRN2", target_bir_lowering=False, debug=False)
    f32 = mybir.dt.float32
    bf16 = mybir.dt.bfloat16
    Relu = mybir.ActivationFunctionType.Relu
    Sigmoid = mybir.ActivationFunctionType.Sigmoid

    xq_h = nc.dram_tensor("xq", [121, LC], bf16, kind="ExternalInput")
    w0s_h = nc.dram_tensor("w0s", [121, 7 * 128], bf16, kind="ExternalInput")
    w1s_h = nc.dram_tensor("w1s", [121, 7 * 128], bf16, kind="ExternalInput")
    w2s_h = nc.dram_tensor("w2s", [120, 2, 128], bf16, kind="ExternalInput")
    b2sa_h = nc.dram_tensor("b2sa", [128, 1], f32, kind="ExternalInput")
    out_h = nc.dram_tensor("out_shard", [100, LC], f32, kind="ExternalOutput")

    with tile.TileContext(nc) as tc:
        with (
            tc.tile_pool(name="const", bufs=1) as cpool,
            tc.tile_pool(name="h1p", bufs=4) as h1pool,
            tc.tile_pool(name="h2p", bufs=8) as h2pool,
            tc.tile_pool(name="ps", bufs=1, space="PSUM") as pspool,
        ):
            xq = cpool.tile([121, LC], bf16, tag="xq")
            w0s = cpool.tile([121, 7 * 128], bf16, tag="w0s")
            w1s = cpool.tile([121, 7 * 128], bf16, tag="w1s")
            w2s = cpool.tile([120, 2, 128], bf16, tag="w2s")
            b2sa = cpool.tile([128, 1], f32, tag="b2sa")
            outba = cpool.tile([106, LC], f32, tag="outba")

            nc.gpsimd.dma_start(w0s[:], w0s_h[:])
            nc.gpsimd.dma_start(b2sa[:], b2sa_h[:])
            nc.gpsimd.dma_start(xq[:], xq_h[:])
            nc.gpsimd.dma_start(w1s[:], w1s_h[:])
            nc.gpsimd.dma_start(w2s[:], w2s_h[:])

            # PE p-state pre-warm on w0s while the xq/w1s DMAs land
            warm = pspool.tile([128, WCH], f32, tag="pa", bufs=2, name="warm")
            for i in range(24):
                nc.tensor.matmul(warm[0:32, 0:256], w0s[:, 0:32],
                                 w0s[:, 0:256], tile_position=(0, 0))

            # software pipeline over tasks s = 4*chunk + pair
            NT = NCH * NPAIR
            pw_by, h1_by, h2_by = {}, {}, {}
            pa_by, pb_by = {}, {}

            # evac engine per q: 0 = scalar(Act), 1 = vector(DVE)
            EV_H1 = [1, 0, 1, 0]
            EV_H2 = [0, 1, 0, 1]

            def evac(e, out_t, in_t):
                # pure relu: out = max(in, 0)
                if e == 0:
                    nc.scalar.activation(out_t, in_t, Relu)
                else:
                    nc.vector.tensor_scalar_max(out_t, in_t, 0.0)

            for s in range(NT + 3):
                c, q = s // NPAIR, s % NPAIR

                # ---- h1 = relu(pw) for task s-1
                if 0 <= s - 1 < NT:
                    q1 = (s - 1) % NPAIR
                    pw = pw_by[s - 1]
                    h1 = h1pool.tile([128, 2, WCH], bf16, tag="h1",
                                     name=f"h1_{s - 1}")
                    h1_by[s - 1] = h1
                    if q1 < 3:
                        evac(EV_H1[q1], h1[0:121, :, :], pw[0:121, :, :])
                    else:
                        evac(EV_H1[q1], h1[0:121, 0, :], pw[0:121, 0, :])

                # ---- h2 = relu(pw) for task s-2
                if 0 <= s - 2 < NT:
                    q2 = (s - 2) % NPAIR
                    pw = pw_by.pop(s - 2)
                    h2 = h2pool.tile([128, 2, WCH], bf16, tag="h2",
                                     name=f"h2_{s - 2}")
                    h2_by[s - 2] = h2
                    if q2 < 3:
                        evac(EV_H2[q2], h2[0:120, :, :], pw[0:120, :, :])
                    else:
                        evac(EV_H2[q2], h2[0:120, 0, :], pw[0:120, 0, :])

                # ---- L0 for task s (both groups; q3 padded to 121 rows)
                if s < NT:
                    fl = slice(c * WCH, (c + 1) * WCH)
                    pw = pspool.tile([128, 2, WCH], f32, tag="pw", bufs=3,
                                     name=f"pw_{s}")
                    pw_by[s] = pw
                    for t in range(2):
                        g = 2 * q + t
                        if g >= 7:
                            continue
                        nc.tensor.matmul(
                            pw[0:121, t, :],
                            w0s[:, 128 * g:128 * g + 121],
                            xq[:, fl],
                        )

                # ---- L1 for task s-1 (overwrites pw; out padded to 120)
                if 0 <= s - 1 < NT:
                    q1 = (s - 1) % NPAIR
                    pw = pw_by[s - 1]
                    h1 = h1_by.pop(s - 1)
                    for t in range(2):
                        g = 2 * q1 + t
                        if g >= 7:
                            continue
                        nc.tensor.matmul(
                            pw[0:120, t, :],
                            w1s[0:121, 128 * g:128 * g + 120],
                            h1[0:121, t, :],
                        )

                # ---- L2: all 7 groups of chunk c2, batched; two groups
                # accumulate into each 30-wide 32-aligned block of pa
                if 0 <= s - 2 < NT and (s - 2) % NPAIR == NPAIR - 1:
                    c2 = (s - 2) // NPAIR
                    pa = pspool.tile([128, WCH], f32, tag="pa", bufs=2,
                                     name=f"pa_{c2}")
                    for g in (0, 2, 4, 6, 1, 3, 5):
                        sg, tg = 4 * c2 + g // 2, g % 2
                        h2 = h2_by[sg]
                        blk = g // 2
                        nc.tensor.matmul(
                            pa[32 * blk:32 * blk + 30, :],
                            w2s[:, g % 2, 32 * blk:32 * blk + 30],
                            h2[0:120, tg, :],
                            start=(g % 2 == 0),
                            stop=(g % 2 == 1 or g == 6),
                            skip_group_check=True,
                            tile_position=(0, 32 * blk),
                        )
                    for g in range(7):
                        h2_by.pop(4 * c2 + g // 2, None)
                    flc = slice(c2 * WCH, (c2 + 1) * WCH)
                    nc.scalar.activation(outba[0:106, flc], pa[0:106, :],
                                         Sigmoid, bias=b2sa[0:106, :],
                                         scale=1.0)
                    if c2 % 2 == 1:
                        fl2 = slice((c2 - 1) * WCH, (c2 + 1) * WCH)
                        for blk in range(4):
                            lo, n = 30 * blk, min(30, 100 - 30 * blk)
                            nc.gpsimd.dma_start(
                                out_h[lo:lo + n, fl2],
                                outba[32 * blk:32 * blk + n, fl2])

    nc.compile()
    _PROGRAM = nc
    return nc


def kernel(seg_feat, conv_weight, ind):
    from concourse.bass_utils import run_bass_kernel_spmd

    in_maps, _ = _prep_inputs(seg_feat, conv_weight, ind)
    nc = build_program()
    res = run_bass_kernel_spmd(nc, in_maps, list(range(NCORE)))
    out = np.empty((B, K, HW), dtype=np.float32)
    for core in range(NCORE):
        b = core // 2
        lo = (core % 2) * LC
        out[b, :, lo:lo + LC] = res.results[core]["out_shard"]
    return out.reshape(B, K, H, Wd)


# revision 14
# speedup vs baseline: 1.4946x; 1.1347x over previous
"""CondInst dynamic mask head on 8 Trainium2 NeuronCores (v3: all-bf16).

Math per instance i: x_i = [rel_i (2,HW); feats_b (8,HW)],
  h1 = relu(w0_i x_i + b0_i); h2 = relu(w1_i h1 + b1_i);
  out_i = sigmoid(w2_i h2 + b2_i).

rel_i is affine in the shared coords map -> folded into shared
X = [coords/128; feats] with per-instance Ahat_i and bias c0_i.

Measured TRN2 matmul behavior (this silicon):
- moving streams ~1 col/cycle @2.4GHz only when the contraction partition
  count is ~>=120; K<=64 runs at ~half rate (458ns vs 252ns per 512-col MM).
- interleaving matmuls of different K-class/dtype costs ~280ns per switch;
  bf16-K121 next to fp8-DR-K120 is free, but plain-fp8 next to bf16 is not.
- fp8 DoubleRow gives NO streaming speedup here, and its outputs must sit
  at PE column band 0 -> useless for multi-block packing.
- matmuls with <=32-wide outputs at distinct 32-aligned column positions
  overlap each other almost fully.

So: EVERY matmul is plain bf16 with K=120/121 (one shape class):
- L0 per group: stationary [121, 8n+1] (rows 0-9 = Ahat, row 10 = c0 bias
  vs the ones-row of X, col 8n emits a constant 1.0 for L1's bias row),
  moving = X padded to 121 rows.  Group q3 is padded to full height.
- L1 per group: block-diag [121, 8n] with row 120(80->padded) = b1.
- L2 per group: [120, 32] stationaries into 7 32-aligned blocks of two psum
  tiles pA/pB; the 7 matmuls are emitted adjacently per chunk so they
  overlap across column positions.
- Evacuations are pure relu (no bias operand) on Act/DVE (GPSIMD cannot
  read PSUM); sigmoid+b2 on Act from pA/pB into SBUF; batched DMA out.
"""

import os
import sys

import numpy as np

sys.path.insert(0, "/opt/trn_rl_repo")
os.environ.setdefault("MYCRO_LOCAL_CACHE", "1")

B, K, C, H, Wd = 4, 100, 8, 128, 128
HW = H * Wd
LC = HW // 2            # 8192 px per core
WCH = 512               # px per chunk
NCH = LC // WCH         # 16 chunks
NCORE = 8
GS = [15, 15, 15, 15, 15, 15, 10]      # group sizes (7 groups, 100 inst)
GOFF = [0, 15, 30, 45, 60, 75, 90]
NPAIR = 4                              # tasks per chunk: (0,1),(2,3),(4,5),(6,)
# L2 output block of group g: (tile 0=pA/1=pB, partition base)
L2BLK = [(0, 0), (0, 32), (0, 64), (0, 96), (1, 0), (1, 32), (1, 64)]

_PROGRAM = None


def _prep_inputs(seg_feat, conv_weight, ind):
    import ml_dtypes
    bf16 = ml_dtypes.bfloat16

    seg_feat = np.asarray(seg_feat, dtype=np.float32)
    conv_weight = np.asarray(conv_weight, dtype=np.float32)
    ind64 = np.asarray(ind).astype(np.int64)

    cw = conv_weight.reshape(B, -1, HW)
    params = np.take_along_axis(cw, ind64[:, None, :], axis=2)  # [B,P,K]
    params = params.transpose(0, 2, 1)                           # [B,K,P]

    w0 = params[..., 0:80].reshape(B, K, C, C + 2)
    w1 = params[..., 80:144].reshape(B, K, C, C)
    w2 = params[..., 144:152].reshape(B, K, 1, C)
    b0 = params[..., 152:160]
    b1 = params[..., 160:168]
    b2 = params[..., 168:169]

    xi = (ind64 % Wd).astype(np.float32)
    yi = (ind64 // Wd).astype(np.float32)
    loc = np.stack([xi, yi], axis=-1)
    w0r = w0[..., 0:2]
    w0f = w0[..., 2:10]
    ahat = np.concatenate([-w0r, w0f], axis=-1)                  # [B,K,8,10]
    c0 = b0 + np.einsum("bkoc,bkc->bko", w0r, loc) / 128.0       # [B,K,8]

    lin = np.arange(HW, dtype=np.float32)
    coords_x = (lin % Wd) / 128.0
    coords_y = np.floor(lin / Wd) / 128.0

    in_maps = []
    for core in range(NCORE):
        b = core // 2
        sl = slice((core % 2) * LC, (core % 2) * LC + LC)

        # ---- xq [121, LC] bf16: L0 moving, padded to the fast K-class
        xq = np.zeros((121, LC), np.float32)
        xq[0] = coords_x[sl]
        xq[1] = coords_y[sl]
        xq[2:10] = seg_feat[b].reshape(C, HW)[:, sl]
        xq[10] = 1.0                     # bias row (c0) + L1 ones source

        # ---- w0s [121, 7*128] bf16 (col 128g+8j+o; col 128g+8n -> 1.0 row)
        w0s = np.zeros((121, 7 * 128), np.float32)
        # ---- w1s [121, 7*128] bf16 (row 8n = b1 ones-row)
        w1s = np.zeros((121, 7 * 128), np.float32)
        # ---- w2s [120, 256] bf16: group g at cols 32g..32g+32
        w2s = np.zeros((120, 2, 128), np.float32)

        for g in range(7):
            n = GS[g]
            for j in range(n):
                i = GOFF[g] + j
                m = 128 * g + 8 * j
                w0s[0:10, m:m + 8] = ahat[b, i].T
                w0s[10, m:m + 8] = c0[b, i]
                w1s[8 * j:8 * j + 8, m:m + 8] = w1[b, i].T
                w1s[8 * n, m:m + 8] = b1[b, i]
                u = 32 * (g // 2) + 15 * (g % 2) + j
                w2s[8 * j:8 * j + 8, g % 2, u] = w2[b, i, 0]
            # constant-1.0 L0 output col (rides the bias row)
            w0s[10, 128 * g + 8 * n] = 1.0

        b2sa = np.zeros((128, 1), np.float32)
        for blk in range(4):
            lo, n = 30 * blk, min(30, 100 - 30 * blk)
            b2sa[32 * blk:32 * blk + n, 0] = b2[b, lo:lo + n, 0]

        in_maps.append({
            "xq": xq.astype(bf16),
            "w0s": w0s.astype(bf16),
            "w1s": w1s.astype(bf16),
            "w2s": w2s.astype(bf16),
            "b2sa": b2sa,
        })

    return in_maps, (b2, np.asarray(ind).dtype)


def build_program():
    global _PROGRAM
    if _PROGRAM is not None:
        return _PROGRAM

    import concourse.tile as tile
    from concourse import bacc, mybir

    nc = bacc.Bacc("TRN2", target_bir_lowering=False, debug=False)
    f32 = mybir.dt.float32
    bf16 = mybir.dt.bfloat16
    Relu = mybir.ActivationFunctionType.Relu
    Sigmoid = mybir.ActivationFunctionType.Sigmoid

    xq_h = nc.dram_tensor("xq", [121, LC], bf16, kind="ExternalInput")
    w0s_h = nc.dram_tensor("w0s", [121, 7 * 128], bf16, kind="ExternalInput")
    w1s_h = nc.dram_tensor("w1s", [121, 7 * 128], bf16, kind="ExternalInput")
    w2s_h = nc.dram_tensor("w2s", [120, 2, 128], bf16, kind="ExternalInput")
    b2sa_h = nc.dram_tensor("b2sa", [128, 1], f32, kind="ExternalInput")
    out_h = nc.dram_tensor("out_shard", [100, LC], f32, kind="ExternalOutput")

    with tile.TileContext(nc) as tc:
        with (
            tc.tile_pool(name="const", bufs=1) as cpool,
            tc.tile_pool(name="h1p", bufs=4) as h1pool,
            tc.tile_pool(name="h2p", bufs=8) as h2pool,
            tc.tile_pool(name="ps", bufs=1, space="PSUM") as pspool,
        ):
            xq = cpool.tile([121, LC], bf16, tag="xq")
            w0s = cpool.tile([121, 7 * 128], bf16, tag="w0s")
            w1s = cpool.tile([121, 7 * 128], bf16, tag="w1s")
            w2s = cpool.tile([120, 2, 128], bf16, tag="w2s")
            b2sa = cpool.tile([128, 1], f32, tag="b2sa")
            outba = cpool.tile([106, LC], f32, tag="outba")

            nc.gpsimd.dma_start(w0s[:], w0s_h[:])
            nc.gpsimd.dma_start(b2sa[:], b2sa_h[:])
            nc.gpsimd.dma_start(xq[:], xq_h[:])
            nc.gpsimd.dma_start(w1s[:], w1s_h[:])
            nc.gpsimd.dma_start(w2s[:], w2s_h[:])

            # PE p-state pre-warm on w0s while the xq/w1s DMAs land
            warm = pspool.tile([128, WCH], f32, tag="pa", bufs=1, name="warm")
            for i in range(24):
                nc.tensor.matmul(warm[0:32, 0:256], w0s[:, 0:32],
                                 w0s[:, 0:256], tile_position=(0, 0))

            # software pipeline over tasks s = 4*chunk + pair
            NT = NCH * NPAIR
            pw_by, h1_by, h2_by, p1_by = {}, {}, {}, {}

            # evac engine per q: 0 = scalar(Act), 1 = vector(DVE)
            EV_H1 = [1, 0, 1, 0]
            EV_H2 = [0, 1, 0, 1]

            def evac(e, out_t, in_t):
                # pure relu: out = max(in, 0)
                if e == 0:
                    nc.scalar.activation(out_t, in_t, Relu)
                else:
                    nc.vector.tensor_scalar_max(out_t, in_t, 0.0)

            for s in range(NT + 5):
                c, q = s // NPAIR, s % NPAIR

                # ---- h1 = relu(pw0) for task s-1 (pair op)
                if 0 <= s - 1 < NT:
                    q1 = (s - 1) % NPAIR
                    pw = pw_by[s - 1]
                    h1 = h1pool.tile([128, 2, WCH], bf16, tag="h1",
                                     name=f"h1_{s - 1}")
                    h1_by[s - 1] = h1
                    if q1 < 3:
                        evac(EV_H1[q1], h1[0:121, :, :], pw[0:121, :, :])
                    else:
                        evac(EV_H1[q1], h1[0:121, 0, :], pw[0:121, 0, :])

                # ---- h2 = relu(pw1) for task s-3 (per-group ops)
                if 0 <= s - 3 < NT:
                    q2 = (s - 3) % NPAIR
                    h2 = h2pool.tile([128, 2, WCH], bf16, tag="h2",
                                     name=f"h2_{s - 3}")
                    h2_by[s - 3] = h2
                    for t in range(2):
                        g = 2 * q2 + t
                        if g >= 7:
                            continue
                        p1g = p1_by.pop((s - 3, t))
                        evac((EV_H2[q2] + t) % 2, h2[0:120, t, :],
                             p1g[0:120, :])

                # ---- L0 for task s (both groups; q3 padded to 121 rows)
                if s < NT:
                    fl = slice(c * WCH, (c + 1) * WCH)
                    pw = pspool.tile([128, 2, WCH], f32, tag="pw", bufs=2,
                                     name=f"pw_{s}")
                    pw_by[s] = pw
                    for t in range(2):
                        g = 2 * q + t
                        if g >= 7:
                            continue
                        nc.tensor.matmul(
                            pw[0:121, t, :],
                            w0s[:, 128 * g:128 * g + 121],
                            xq[:, fl],
                        )

                # ---- L1 for task s-2 (per-group psum tiles, lag 2)
                if 0 <= s - 2 < NT:
                    q1 = (s - 2) % NPAIR
                    pw_by.pop(s - 2, None)
                    h1 = h1_by.pop(s - 2)
                    for t in range(2):
                        g = 2 * q1 + t
                        if g >= 7:
                            continue
                        p1g = pspool.tile([128, WCH], f32, tag="p1", bufs=3,
                                          name=f"p1_{s - 2}_{t}")
                        p1_by[(s - 2, t)] = p1g
                        nc.tensor.matmul(
                            p1g[0:120, :],
                            w1s[0:121, 128 * g:128 * g + 120],
                            h1[0:121, t, :],
                        )

                # ---- L2: all 7 groups of chunk c2, batched; two groups
                # accumulate into each 30-wide 32-aligned block of pa
                if 0 <= s - 3 < NT and (s - 3) % NPAIR == NPAIR - 1:
                    c2 = (s - 3) // NPAIR
                    pa = pspool.tile([128, WCH], f32, tag="pa", bufs=1,
                                     name=f"pa_{c2}")
                    for g in (0, 2, 4, 6, 1, 3, 5):
                        sg, tg = 4 * c2 + g // 2, g % 2
                        h2 = h2_by[sg]
                        blk = g // 2
                        nc.tensor.matmul(
                            pa[32 * blk:32 * blk + 30, :],
                            w2s[:, g % 2, 32 * blk:32 * blk + 30],
                            h2[0:120, tg, :],
                            start=(g % 2 == 0),
                            stop=(g % 2 == 1 or g == 6),
                            skip_group_check=True,
                            tile_position=(0, 32 * blk),
                        )
                    for g in range(7):
                        h2_by.pop(4 * c2 + g // 2, None)
                    flc = slice(c2 * WCH, (c2 + 1) * WCH)
                    nc.scalar.activation(outba[0:106, flc], pa[0:106, :],
                                         Sigmoid, bias=b2sa[0:106, :],
                                         scale=1.0)
                    if c2 % 2 == 1:
                        fl2 = slice((c2 - 1) * WCH, (c2 + 1) * WCH)
                        for blk in range(4):
                            lo, n = 30 * blk, min(30, 100 - 30 * blk)
                            nc.gpsimd.dma_start(
                                out_h[lo:lo + n, fl2],
                                outba[32 * blk:32 * blk + n, fl2])

    nc.compile()
    _PROGRAM = nc
    return nc


def kernel(seg_feat, conv_weight, ind):
    from concourse.bass_utils import run_bass_kernel_spmd

    in_maps, _ = _prep_inputs(seg_feat, conv_weight, ind)
    nc = build_program()
    res = run_bass_kernel_spmd(nc, in_maps, list(range(NCORE)))
    out = np.empty((B, K, HW), dtype=np.float32)
    for core in range(NCORE):
        b = core // 2
        lo = (core % 2) * LC
        out[b, :, lo:lo + LC] = res.results[core]["out_shard"]
    return out.reshape(B, K, H, Wd)
